# revision 13
# baseline (speedup 1.0000x reference)
"""Capsule-routing kernel for Trainium2 (8 NeuronCores, data-parallel over batch).

Reference computation (per batch element):
    u_hat[i, (n,d)] = sum_e u_vecs[i,e] * W[e, n*32+d]        # (In=2048, 512)
    b = 0
    for it in 3:
        c = softmax(b, axis=n)                                 # (16, 2048)
        temp1 = target * c
        s[n,d] = sum_i temp1[n,i] * u_hat[i, n*32+d]
        outputs = s / sqrt(sum_d s^2 + 1e-7)
        if it < 2: b[n,i] = sum_d outputs[n,d] * u_hat[i, n*32+d]

Key refactor: u_hat is never materialized.  With
    v[n,e]  = sum_i temp1[n,i] * u_vecs[i,e]                   # (16, 256)
    s[n,d]  = sum_e v[n,e] * W[e, n*32+d]
    WO[n,e] = sum_d outputs[n,d] * W[e, n*32+d]                # (16, 256)
    b[n,i]  = sum_e WO[n,e] * u_vecs[i,e]
all large operands (u_vecs natural + transposed layouts) stay resident in SBUF
and DRAM traffic is just the initial input load (~9 MB/core).
"""

import sys

if "/opt/trn_rl_repo" not in sys.path:
    sys.path.insert(0, "/opt/trn_rl_repo")

import numpy as np

import concourse.bacc as bacc
import concourse.bass as bass
import concourse.tile as tile
from concourse import mybir
from concourse.masks import make_identity

F32 = mybir.dt.float32
P = 128
B_LOC = 4       # batch elements per core (32 / 8 cores)
IN = 2048       # num input capsules
DIN = 256       # input capsule dim (2 chunks of 128)
NCAP = 16       # num output capsules
DCAP = 32       # output capsule dim
T = IN // P     # 16 i-tiles
NJ = DIN // P   # 2 e-chunks
EPS = 1e-7


def build_body(tc, o_ap, u_ap, tg_ap, w_ap):
    from contextlib import ExitStack

    nc = tc.nc
    ctx = ExitStack()

    const = ctx.enter_context(tc.tile_pool(name="const", bufs=1))
    sb_u = ctx.enter_context(tc.tile_pool(name="sbu", bufs=1))
    sb_tgn = ctx.enter_context(tc.tile_pool(name="tgn", bufs=2))
    work = ctx.enter_context(tc.tile_pool(name="work", bufs=3))
    small = ctx.enter_context(tc.tile_pool(name="small", bufs=4))
    wot_pool = ctx.enter_context(tc.tile_pool(name="wot", bufs=4))
    ps_stage = ctx.enter_context(tc.tile_pool(name="pstage", bufs=2, space="PSUM"))
    ps_bt = ctx.enter_context(tc.tile_pool(name="psbt", bufs=2, space="PSUM"))
    ps_small = ctx.enter_context(tc.tile_pool(name="pssmall", bufs=4, space="PSUM"))

    identity = const.tile([P, P], F32)
    make_identity(nc, identity)

    # eps bias tiles for the squash sqrt (iter0 uses 256*eps: unscaled target)
    eps0 = const.tile([NCAP, 1], F32)
    nc.vector.memset(eps0, EPS * (NCAP * NCAP))
    eps1 = const.tile([NCAP, 1], F32)
    nc.vector.memset(eps1, EPS)

    # Block-diagonal mask: mask[n, (n', d)] = 1.0 iff n == n'
    mask = const.tile([NCAP, NCAP * DCAP], F32)
    nc.gpsimd.memset(mask, 0.0)
    nc.gpsimd.affine_select(
        out=mask.rearrange("p (a b) -> p a b", b=DCAP),
        in_=mask.rearrange("p (a b) -> p a b", b=DCAP),
        compare_op=mybir.AluOpType.not_equal,
        fill=1.0,
        base=0,
        pattern=[[-1, NCAP], [0, DCAP]],
        channel_multiplier=1,
    )

    # W natural: W_sb[p, j, f] = W[j*128+p, f]
    w_sb = const.tile([P, NJ, 512], F32)
    nc.sync.dma_start(out=w_sb, in_=w_ap.rearrange("(j p) f -> p j f", p=P))

    # W transposed: wt_sb[p, jj, e] = W[e, jj*128+p]
    wt_sb = const.tile([P, 4, DIN], F32)
    for jj in range(4):
        stg = ps_stage.tile([P, 512], F32, tag="stage")
        for k in range(NJ):
            nc.tensor.transpose(
                stg[:, k * P:(k + 1) * P], w_sb[:, k, jj * P:(jj + 1) * P], identity
            )
        nc.vector.tensor_copy(out=wt_sb[:, jj, :], in_=stg[:, :DIN])

    u_sb = []    # natural: u_sb[b][p, t, e] = u_vecs[b, t*128+p, e]
    ut_sb = []   # transposed: ut_sb[b][p, j, i] = u_vecs[b, i, j*128+p]
    tt_sb = []   # targetT: tt_sb[b][p, t*16+n] = target[b, n, t*128+p]
    for b in range(B_LOC):
        ub = sb_u.tile([P, T, DIN], F32, tag=f"u{b}")
        src = u_ap[b].rearrange("(t p) e -> p t e", p=P)
        for g in range(4):
            nc.sync.dma_start(out=ub[:, 4 * g:4 * g + 4, :], in_=src[:, 4 * g:4 * g + 4, :])
        u_sb.append(ub)

        tgn = sb_tgn.tile([NCAP, IN], F32, tag="tgn")
        nc.sync.dma_start(out=tgn, in_=tg_ap[b])
        ttb = sb_u.tile([P, T * NCAP], F32, tag=f"tt{b}")
        stg = ps_bt.tile([P, T * NCAP], F32, tag="bt")
        for t in range(T):
            nc.tensor.transpose(
                stg[:, t * NCAP:(t + 1) * NCAP],
                tgn[:, t * P:(t + 1) * P],
                identity[:NCAP, :NCAP],
            )
        nc.scalar.copy(out=ttb, in_=stg)
        tt_sb.append(ttb)

    for b in range(B_LOC):
        utb = sb_u.tile([P, NJ, IN], F32, tag=f"ut{b}")
        for j in range(NJ):
            for q in range(4):
                stg = ps_stage.tile([P, 512], F32, tag="stage")
                for dt in range(4):
                    t = 4 * q + dt
                    nc.tensor.transpose(
                        stg[:, dt * P:(dt + 1) * P],
                        u_sb[b][:, t, j * P:(j + 1) * P],
                        identity,
                    )
                if (j * 4 + q) % 2 == 0:
                    nc.vector.tensor_copy(out=utb[:, j, q * 512:(q + 1) * 512], in_=stg)
                else:
                    nc.scalar.copy(out=utb[:, j, q * 512:(q + 1) * 512], in_=stg)
        ut_sb.append(utb)

    wot_prev = [None] * B_LOC
    for it in range(3):
        for b in range(B_LOC):
            if it == 0:
                # b == 0 -> c = 1/16 uniform; squash is scale-invariant so use
                # target directly as temp1 (eps correction folded into sqrt bias).
                t1 = tt_sb[b]
                sqrt_bias = eps0
            else:
                # --- b-update: bT[i, n] = sum_e uT[e,i] * WOT[e,n] ---
                bt_ps = ps_bt.tile([P, T * NCAP], F32, tag="bt")
                for t in range(T):
                    for j in range(NJ):
                        nc.tensor.matmul(
                            bt_ps[:, t * NCAP:(t + 1) * NCAP],
                            lhsT=ut_sb[b][:, j, t * P:(t + 1) * P],
                            rhs=wot_prev[b][:, j * NCAP:(j + 1) * NCAP],
                            start=(j == 0),
                            stop=(j == NJ - 1),
                        )
                # --- softmax over n (free-dim groups of 16) + target modulate ---
                e_sb = work.tile([P, T * NCAP], F32, tag="esb")
                nc.scalar.activation(e_sb, bt_ps, mybir.ActivationFunctionType.Exp)
                den = small.tile([P, T], F32, tag="den")
                nc.vector.reduce_sum(
                    den, e_sb.rearrange("p (t n) -> p t n", n=NCAP),
                    axis=mybir.AxisListType.X,
                )
                rden = small.tile([P, T], F32, tag="rden")
                nc.vector.reciprocal(rden, den)
                t1 = work.tile([P, T * NCAP], F32, tag="t1")
                rden_bc = bass.AP(
                    tensor=rden.tensor,
                    offset=rden.offset,
                    ap=[rden.ap[0], list(rden.ap[1]), [0, NCAP]],
                )
                tmp = work.tile([P, T * NCAP], F32, tag="tmp")
                nc.vector.tensor_mul(tmp, e_sb, tt_sb[b])
                nc.vector.tensor_tensor(
                    out=t1.rearrange("p (t n) -> p t n", n=NCAP),
                    in0=tmp.rearrange("p (t n) -> p t n", n=NCAP),
                    in1=rden_bc,
                    op=mybir.AluOpType.mult,
                )
                sqrt_bias = eps1

            # --- v: vT[e, n] = sum_i u[i, e] * temp1T[i, n] ---
            vt_ps = ps_small.tile([P, NJ * NCAP], F32, tag="psm")
            for j in range(NJ):
                for t in range(T):
                    nc.tensor.matmul(
                        vt_ps[:, j * NCAP:(j + 1) * NCAP],
                        lhsT=u_sb[b][:, t, j * P:(j + 1) * P],
                        rhs=t1[:, t * NCAP:(t + 1) * NCAP],
                        start=(t == 0),
                        stop=(t == T - 1),
                    )
            vt = work.tile([P, NJ * NCAP], F32, tag="vt")
            nc.vector.tensor_copy(out=vt, in_=vt_ps)

            # --- sfull[m, (n,d)] = sum_e v[m, e] * W[e, (n,d)]; diag blocks are s ---
            s_ps = ps_small.tile([NCAP, NCAP * DCAP], F32, tag="psm")
            for j in range(NJ):
                nc.tensor.matmul(
                    s_ps,
                    lhsT=vt[:, j * NCAP:(j + 1) * NCAP],
                    rhs=w_sb[:, j, :],
                    start=(j == 0),
                    stop=(j == NJ - 1),
                )
            # --- squash on the block-diag masked sfull ---
            masked = work.tile([NCAP, NCAP * DCAP], F32, tag="masked")
            nc.vector.tensor_mul(masked, s_ps, mask)
            sq = work.tile([NCAP, NCAP * DCAP], F32, tag="sq")
            ss = small.tile([NCAP, 1], F32, tag="ss")
            nc.scalar.activation(sq, masked, mybir.ActivationFunctionType.Square,
                                 accum_out=ss)
            ssq = small.tile([NCAP, 1], F32, tag="ssq")
            nc.scalar.activation(ssq, ss, mybir.ActivationFunctionType.Sqrt,
                                 bias=sqrt_bias)
            rinv = small.tile([NCAP, 1], F32, tag="rinv")
            nc.vector.reciprocal(rinv, ssq)
            # bdT[n, (n',d)] = outputs[n,d] iff n'==n  (block-diag O^T, free)
            bdT = work.tile([NCAP, NCAP * DCAP], F32, tag="bdT")
            nc.vector.tensor_scalar_mul(bdT, masked, rinv)

            if it == 2:
                # compact outputs: sum over n' of the block-diag rows
                outp = work.tile([NCAP, DCAP], F32, tag="outp")
                nc.vector.reduce_sum(
                    outp,
                    bass.AP(
                        tensor=bdT.tensor,
                        offset=bdT.offset,
                        ap=[bdT.ap[0], [1, DCAP], [DCAP, NCAP]],
                    ),
                    axis=mybir.AxisListType.X,
                )
                nc.sync.dma_start(out=o_ap[b], in_=outp)
            else:
                obd_ps = ps_small.tile([P, 4 * NCAP], F32, tag="psm")
                for q in range(4):
                    nc.tensor.transpose(
                        obd_ps[:, q * NCAP:(q + 1) * NCAP],
                        bdT[:, q * P:(q + 1) * P],
                        identity[:NCAP, :NCAP],
                    )
                obd = work.tile([P, 4 * NCAP], F32, tag="obd")
                nc.scalar.copy(out=obd, in_=obd_ps)
                # WO[n, e] = sum_{(n',d)} O_bd[(n',d), n] * WT[(n',d), e]
                wo_ps = ps_small.tile([NCAP, DIN], F32, tag="psm")
                for q in range(4):
                    nc.tensor.matmul(
                        wo_ps,
                        lhsT=obd[:, q * NCAP:(q + 1) * NCAP],
                        rhs=wt_sb[:, q, :],
                        start=(q == 0),
                        stop=(q == 3),
                    )
                wo = work.tile([NCAP, DIN], F32, tag="wo")
                nc.vector.tensor_copy(out=wo, in_=wo_ps)
                # WOT chunks: woT[e(part), j*16+n] = WO[n, j*128+e]
                wot_ps = ps_small.tile([P, NJ * NCAP], F32, tag="psm")
                for j in range(NJ):
                    nc.tensor.transpose(
                        wot_ps[:, j * NCAP:(j + 1) * NCAP],
                        wo[:, j * P:(j + 1) * P],
                        identity[:NCAP, :NCAP],
                    )
                wot = wot_pool.tile([P, NJ * NCAP], F32, tag="wot")
                nc.vector.tensor_copy(out=wot, in_=wot_ps)
                wot_prev[b] = wot

    ctx.close()


def build_nc():
    nc = bacc.Bacc("TRN2")
    u = nc.dram_tensor("u_vecs", [B_LOC, IN, DIN], F32, kind="ExternalInput").ap()
    tg = nc.dram_tensor("target", [B_LOC, NCAP, IN], F32, kind="ExternalInput").ap()
    w = nc.dram_tensor("W", [DIN, NCAP * DCAP], F32, kind="ExternalInput").ap()
    o = nc.dram_tensor("out", [B_LOC, NCAP, DCAP], F32, kind="ExternalOutput").ap()
    with tile.TileContext(nc) as tc:
        build_body(tc, o, u, tg, w)
    nc.compile()
    return nc


_NC_CACHE = None


def kernel(u_vecs, target, W, _trace=False, **_trace_kwargs):
    global _NC_CACHE
    from concourse.bass_utils import run_bass_kernel_spmd

    if _NC_CACHE is None:
        _NC_CACHE = build_nc()
    nc = _NC_CACHE

    n_cores = 8
    in_maps = []
    for c in range(n_cores):
        sl = slice(c * B_LOC, (c + 1) * B_LOC)
        in_maps.append({
            "u_vecs": np.ascontiguousarray(u_vecs[sl]),
            "target": np.ascontiguousarray(target[sl]),
            "W": np.ascontiguousarray(W),
        })
    res = run_bass_kernel_spmd(nc, in_maps, list(range(n_cores)),
                               trace=_trace, **_trace_kwargs)
    out = np.concatenate([res.results[c]["out"] for c in range(n_cores)], axis=0)
    if _trace:
        return out, res
    return out


if __name__ == "__main__":
    rng = np.random.default_rng(0)
    u = rng.standard_normal((32, IN, DIN), dtype=np.float32)
    t = rng.random((32, NCAP, IN), dtype=np.float32)
    w = rng.standard_normal((DIN, NCAP * DCAP), dtype=np.float32) * 0.06
    print(kernel(u, t, w).shape)


# revision 43
# speedup vs baseline: 89.3924x; 89.3924x over previous
"""Capsule-routing kernel for Trainium2 (8 NeuronCores, data-parallel over batch).

Reference computation (per batch element):
    u_hat[i, (n,d)] = sum_e u_vecs[i,e] * W[e, n*32+d]        # (In=2048, 512)
    b = 0
    for it in 3:
        c = softmax(b, axis=n)                                 # (16, 2048)
        temp1 = target * c
        s[n,d] = sum_i temp1[n,i] * u_hat[i, n*32+d]
        outputs = s / sqrt(sum_d s^2 + 1e-7)
        if it < 2: b[n,i] = sum_d outputs[n,d] * u_hat[i, n*32+d]

Key refactor: u_hat is never materialized.  With
    v[n,e]  = sum_i temp1[n,i] * u_vecs[i,e]                   # (16, 256)
    s[n,d]  = sum_e v[n,e] * W[e, n*32+d]
    WO[n,e] = sum_d outputs[n,d] * W[e, n*32+d]                # (16, 256)
    b[n,i]  = sum_e WO[n,e] * u_vecs[i,e]
all large operands (u_vecs natural + transposed layouts) stay resident in SBUF
and DRAM traffic is just the initial input load (~9 MB/core).

Batches are processed in pairs, column/partition-packed, so each small
DVE/ACT op and PSUM tile covers two batch elements.
"""

import sys

if "/opt/trn_rl_repo" not in sys.path:
    sys.path.insert(0, "/opt/trn_rl_repo")

import numpy as np

import concourse.bacc as bacc
import concourse.bass as bass
import concourse.tile as tile
from concourse import mybir
from concourse.masks import make_identity

F32 = mybir.dt.float32
P = 128
B_LOC = 4       # batch elements per core (32 / 8 cores)
IN = 2048       # num input capsules
DIN = 256       # input capsule dim (2 chunks of 128)
NCAP = 16       # num output capsules
DCAP = 32       # output capsule dim
T = IN // P     # 16 i-tiles
NJ = DIN // P   # 2 e-chunks
EPS = 1e-7
F = NCAP * DCAP  # 512


def build_body(tc, o_ap, u_ap, tg_ap, w_ap):
    from contextlib import ExitStack

    nc = tc.nc
    ctx = ExitStack()

    const = ctx.enter_context(tc.tile_pool(name="const", bufs=1))
    sb_u = ctx.enter_context(tc.tile_pool(name="sbu", bufs=1))
    sb_tgn = ctx.enter_context(tc.tile_pool(name="tgn", bufs=2))
    work = ctx.enter_context(tc.tile_pool(name="work", bufs=3))
    small = ctx.enter_context(tc.tile_pool(name="small", bufs=4))
    wot_pool = ctx.enter_context(tc.tile_pool(name="wot", bufs=3))
    ps_bt = ctx.enter_context(tc.tile_pool(name="psbt", bufs=2, space="PSUM"))
    ps_small = ctx.enter_context(tc.tile_pool(name="pssmall", bufs=6, space="PSUM"))
    ps_stage = ps_bt

    BF16 = mybir.dt.bfloat16
    identity = const.tile([P, P], F32)
    make_identity(nc, identity)
    identity_bf = const.tile([P, P], BF16)
    make_identity(nc, identity_bf)

    # eps bias tiles for the squash sqrt (iter0 uses 256*eps: unscaled target)
    eps0 = const.tile([2 * NCAP, 1], F32)
    nc.vector.memset(eps0, EPS * (NCAP * NCAP))
    eps1 = const.tile([2 * NCAP, 1], F32)
    nc.vector.memset(eps1, EPS)

    # Block-diagonal mask for batch pairs: mask32[(b',n), (n',d)] = 1 iff n'==n
    mask16 = const.tile([NCAP, F], F32)
    nc.gpsimd.memset(mask16, 0.0)
    nc.gpsimd.affine_select(
        out=mask16.rearrange("p (a b) -> p a b", b=DCAP),
        in_=mask16.rearrange("p (a b) -> p a b", b=DCAP),
        compare_op=mybir.AluOpType.not_equal,
        fill=1.0,
        base=0,
        pattern=[[-1, NCAP], [0, DCAP]],
        channel_multiplier=1,
    )
    mask32 = const.tile([2 * NCAP, F], F32)
    nc.sync.dma_start(out=mask32[:NCAP, :], in_=mask16)
    nc.sync.dma_start(out=mask32[NCAP:, :], in_=mask16)

    # W natural: W_sb[p, j, f] = W[j*128+p, f]
    w_sb = const.tile([P, NJ, F], F32)
    nc.sync.dma_start(out=w_sb, in_=w_ap.rearrange("(j p) f -> p j f", p=P))

    # W transposed: wt_sb[p, jj, e] = W[e, jj*128+p]
    wt_sb = const.tile([P, 4, DIN], F32)
    for jj in range(4):
        stg = ps_stage.tile([P, 512], F32, tag="bt")
        for k in range(NJ):
            nc.tensor.transpose(
                stg[:, k * P:(k + 1) * P], w_sb[:, k, jj * P:(jj + 1) * P], identity
            )
        nc.vector.tensor_copy(out=wt_sb[:, jj, :], in_=stg[:, :DIN])

    u_sb = []    # natural: u_sb[b][p, t, e] = u_vecs[b, t*128+p, e]
    ut_sb = []   # transposed: ut_sb[b][p, j, i] = u_vecs[b, i, j*128+p]
    tt_sb = []   # targetT pairs: tt_sb[k][p, b', t*16+n] = target[2k+b', n, t*128+p]
    for b in range(B_LOC):
        ub = sb_u.tile([P, T, DIN], F32, tag=f"u{b}")
        src = u_ap[b].rearrange("(t p) e -> p t e", p=P)
        for g in range(4):
            nc.sync.dma_start(out=ub[:, 4 * g:4 * g + 4, :],
                              in_=src[:, 4 * g:4 * g + 4, :])
        u_sb.append(ub)

    for k in range(B_LOC // 2):
        stg = ps_bt.tile([P, 2 * T * NCAP], F32, tag="bt")
        for bp in range(2):
            tgn = sb_tgn.tile([NCAP, IN], F32, tag="tgn")
            nc.sync.dma_start(out=tgn, in_=tg_ap[2 * k + bp])
            for t in range(T):
                nc.tensor.transpose(
                    stg[:, bp * 256 + t * NCAP: bp * 256 + (t + 1) * NCAP],
                    tgn[:, t * P:(t + 1) * P],
                    identity[:NCAP, :NCAP],
                )
        ttk = sb_u.tile([P, 2, T * NCAP], F32, tag=f"tt{k}")
        nc.scalar.copy(out=ttk.rearrange("p a b -> p (a b)"), in_=stg)
        tt_sb.append(ttk)

    def build_ut(b):
        # bf16 copy of u (only feeds the b-update routing logits), then
        # PE-transpose bf16 blocks: FWL halves LDWEIGHTS cost.
        ubf = sb_tgn.tile([P, T * DIN], BF16, tag="ubf")
        if b % 2 == 0:
            nc.vector.tensor_copy(out=ubf, in_=u_sb[b].rearrange("p t e -> p (t e)"))
        else:
            nc.scalar.copy(out=ubf, in_=u_sb[b].rearrange("p t e -> p (t e)"))
        utb = sb_u.tile([P, NJ, IN], BF16, tag=f"ut{b}")
        for j in range(NJ):
            for q in range(4):
                stg = ps_stage.tile([P, 512], BF16, tag="bt")
                for dt in range(4):
                    t = 4 * q + dt
                    nc.tensor.transpose(
                        stg[:, dt * P:(dt + 1) * P],
                        ubf[:, t * DIN + j * P: t * DIN + (j + 1) * P],
                        identity_bf,
                    )
                if (j * 4 + q) % 2 == 0:
                    nc.vector.tensor_copy(out=utb[:, j, q * 512:(q + 1) * 512],
                                          in_=stg)
                else:
                    nc.scalar.copy(out=utb[:, j, q * 512:(q + 1) * 512], in_=stg)
        return utb

    wot_prev = [None, None]  # per pair

    def iter_block(it, k):
        if it == 0:
            # b == 0 -> c = 1/16 uniform; squash is scale-invariant so use
            # target directly as temp1 (eps correction folded into sqrt bias).
            t1 = tt_sb[k]
            eps = eps0
        else:
            # --- b-update: bT[i, (b',n)] = sum_e uT[e,i] * WOT[e,(b',n)] ---
            bt_ps = ps_bt.tile([P, 2 * T * NCAP], F32, tag="bt")
            wot = wot_prev[k]
            for bp in range(2):
                for t in range(T):
                    for j in range(NJ):
                        nc.tensor.matmul(
                            bt_ps[:, bp * 256 + t * NCAP: bp * 256 + (t + 1) * NCAP],
                            lhsT=ut_sb[2 * k + bp][:, j, t * P:(t + 1) * P],
                            rhs=wot[:, j, bp, :],
                            start=(j == 0),
                            stop=(j == NJ - 1),
                        )
            # --- softmax over n (free groups of 16) + target modulate ---
            e_sb = work.tile([P, 2 * T * NCAP], F32, tag="esb")
            nc.scalar.activation(e_sb, bt_ps, mybir.ActivationFunctionType.Exp)
            den = small.tile([P, 2, T], F32, tag="den")
            nc.vector.reduce_sum(
                den.rearrange("p a t -> p (a t)"),
                e_sb.rearrange("p (a t n) -> p a t n", t=T, n=NCAP),
                axis=mybir.AxisListType.X,
            )
            rden = small.tile([P, 2, T], F32, tag="rden")
            nc.vector.reciprocal(rden.rearrange("p a t -> p (a t)"),
                                 den.rearrange("p a t -> p (a t)"))
            tmp = work.tile([P, 2 * T * NCAP], F32, tag="tmp")
            nc.vector.tensor_mul(tmp, e_sb, tt_sb[k].rearrange("p a b -> p (a b)"))
            t1 = work.tile([P, 2, T * NCAP], F32, tag="t1")
            rden_bc = bass.AP(
                tensor=rden.tensor,
                offset=rden.offset,
                ap=[rden.ap[0], [T, 2], [1, T], [0, NCAP]],
            )
            nc.vector.tensor_tensor(
                out=t1.rearrange("p a (t n) -> p a t n", n=NCAP),
                in0=tmp.rearrange("p (a t n) -> p a t n", t=T, n=NCAP),
                in1=rden_bc,
                op=mybir.AluOpType.mult,
            )
            eps = eps1

        # --- v[(b',n), e] = sum_i temp1T[i, (b',n)] * u[i, e] ---
        v_ps = ps_small.tile([NCAP, 2, DIN], F32, tag="psm")
        for bp in range(2):
            for t in range(T):
                nc.tensor.matmul(
                    v_ps[:, bp, :],
                    lhsT=t1[:, bp, t * NCAP:(t + 1) * NCAP],
                    rhs=u_sb[2 * k + bp][:, t, :],
                    start=(t == 0),
                    stop=(t == T - 1),
                )
        v_sb = work.tile([NCAP, 2, DIN], F32, tag="vsb")
        nc.scalar.copy(out=v_sb.rearrange("p a b -> p (a b)"),
                       in_=v_ps.rearrange("p a b -> p (a b)"))
        # vT: [e(part), j, b', n]
        vt_ps = ps_small.tile([P, NJ, 2, NCAP], F32, tag="psm")
        for j in range(NJ):
            for bp in range(2):
                nc.tensor.transpose(
                    vt_ps[:, j, bp, :],
                    v_sb[:, bp, j * P:(j + 1) * P],
                    identity[:NCAP, :NCAP],
                )
        vt = work.tile([P, NJ, 2, NCAP], F32, tag="vt")
        nc.vector.tensor_copy(out=vt.rearrange("p a b c -> p (a b c)"),
                              in_=vt_ps.rearrange("p a b c -> p (a b c)"))

        # --- sfull[(b',m), (n,d)] = sum_e v[(b',m), e] * W[e, (n,d)] ---
        s_ps = ps_small.tile([2 * NCAP, F], F32, tag="psm")
        for j in range(NJ):
            nc.tensor.matmul(
                s_ps,
                lhsT=vt[:, j, :, :],
                rhs=w_sb[:, j, :],
                start=(j == 0),
                stop=(j == NJ - 1),
            )
        # --- squash via block-diag mask (both batches at once) ---
        masked = work.tile([2 * NCAP, F], F32, tag="masked")
        nc.vector.tensor_mul(masked, s_ps, mask32)
        sq = work.tile([2 * NCAP, F], F32, tag="tmp")
        ss = small.tile([2 * NCAP, 1], F32, tag="ss")
        nc.scalar.activation(sq, masked, mybir.ActivationFunctionType.Square,
                             accum_out=ss)
        ssq = small.tile([2 * NCAP, 1], F32, tag="ssq")
        nc.scalar.activation(ssq, ss, mybir.ActivationFunctionType.Sqrt, bias=eps)
        rinv = small.tile([2 * NCAP, 1], F32, tag="rinv")
        nc.vector.reciprocal(rinv, ssq)
        # bdT[(b',n), (n',d)] = outputs[b',n,d] iff n'==n
        bdT = work.tile([2 * NCAP, F], F32, tag="bdT")
        nc.vector.tensor_scalar_mul(bdT, masked, rinv)

        if it == 2:
            outp = work.tile([2 * NCAP, DCAP], F32, tag="outp")
            nc.vector.reduce_sum(
                outp,
                bass.AP(
                    tensor=bdT.tensor,
                    offset=bdT.offset,
                    ap=[bdT.ap[0], [1, DCAP], [DCAP, NCAP]],
                ),
                axis=mybir.AxisListType.X,
            )
            nc.sync.dma_start(
                out=o_ap[2 * k:2 * k + 2].rearrange("b n d -> (b n) d"),
                in_=outp,
            )
        else:
            # O_bd chunks: transpose bdT q-block (32,128) -> (128, (q,b',n))
            obd_ps = ps_small.tile([P, 4, 2, NCAP], F32, tag="psm")
            for q in range(4):
                nc.tensor.transpose(
                    obd_ps[:, q, :, :],
                    bdT[:, q * P:(q + 1) * P],
                    identity[:2 * NCAP, :2 * NCAP],
                )
            obd = work.tile([P, 4, 2, NCAP], F32, tag="obd")
            nc.scalar.copy(out=obd.rearrange("p a b c -> p (a b c)"),
                           in_=obd_ps.rearrange("p a b c -> p (a b c)"))
            # WO[n, (b',e)] = sum_{(n',d)} O_bd[(n',d), (b',n)] * WT[(n',d), e]
            wo_ps = ps_small.tile([NCAP, 2, DIN], F32, tag="psm")
            for bp in range(2):
                for q in range(4):
                    nc.tensor.matmul(
                        wo_ps[:, bp, :],
                        lhsT=obd[:, q, bp, :],
                        rhs=wt_sb[:, q, :],
                        start=(q == 0),
                        stop=(q == 3),
                    )
            wo = work.tile([NCAP, 2, DIN], F32, tag="wo")
            nc.vector.tensor_copy(out=wo.rearrange("p a b -> p (a b)"),
                                  in_=wo_ps.rearrange("p a b -> p (a b)"))
            # WOT: [e(part), j, b', n]
            wot_ps = ps_small.tile([P, NJ, 2, NCAP], F32, tag="psm")
            for j in range(NJ):
                for bp in range(2):
                    nc.tensor.transpose(
                        wot_ps[:, j, bp, :],
                        wo[:, bp, j * P:(j + 1) * P],
                        identity[:NCAP, :NCAP],
                    )
            wot = wot_pool.tile([P, NJ, 2, NCAP], BF16, tag="wot")
            nc.vector.tensor_copy(out=wot.rearrange("p a b c -> p (a b c)"),
                                  in_=wot_ps.rearrange("p a b c -> p (a b c)"))
            wot_prev[k] = wot

    for k in range(B_LOC // 2):
        iter_block(0, k)
    for b in range(B_LOC):
        ut_sb.append(build_ut(b))
    for it in (1, 2):
        for k in range(B_LOC // 2):
            iter_block(it, k)

    ctx.close()


def build_nc(loop_n=0):
    nc = bacc.Bacc("TRN2")
    u = nc.dram_tensor("u_vecs", [B_LOC, IN, DIN], F32, kind="ExternalInput").ap()
    tg = nc.dram_tensor("target", [B_LOC, NCAP, IN], F32, kind="ExternalInput").ap()
    w = nc.dram_tensor("W", [DIN, NCAP * DCAP], F32, kind="ExternalInput").ap()
    o = nc.dram_tensor("out", [B_LOC, NCAP, DCAP], F32, kind="ExternalOutput").ap()
    with tile.TileContext(nc) as tc:
        if loop_n:
            with tc.For_i(0, loop_n, 1):
                build_body(tc, o, u, tg, w)
        else:
            build_body(tc, o, u, tg, w)
    nc.compile()
    return nc


_NC_CACHE = None


def kernel(u_vecs, target, W, _trace=False, **_trace_kwargs):
    global _NC_CACHE
    from concourse.bass_utils import run_bass_kernel_spmd

    if _NC_CACHE is None:
        _NC_CACHE = build_nc()
    nc = _NC_CACHE

    n_cores = 8
    in_maps = []
    for c in range(n_cores):
        sl = slice(c * B_LOC, (c + 1) * B_LOC)
        in_maps.append({
            "u_vecs": np.ascontiguousarray(u_vecs[sl]),
            "target": np.ascontiguousarray(target[sl]),
            "W": np.ascontiguousarray(W),
        })
    res = run_bass_kernel_spmd(nc, in_maps, list(range(n_cores)),
                               trace=_trace, **_trace_kwargs)
    out = np.concatenate([res.results[c]["out"] for c in range(n_cores)], axis=0)
    if _trace:
        return out, res
    return out


if __name__ == "__main__":
    rng = np.random.default_rng(0)
    u = rng.standard_normal((32, IN, DIN), dtype=np.float32)
    t = rng.random((32, NCAP, IN), dtype=np.float32)
    w = rng.standard_normal((DIN, NCAP * DCAP), dtype=np.float32) * 0.06
    print(kernel(u, t, w).shape)
